# revision 1
# baseline (speedup 1.0000x reference)
"""Trainium2 Bass kernel for DiceFromLabelsLoss (histogram binning).

Strategy: data-parallel over the flattened voxel dim across 8 cores (each
core gets half of one sample). Per core, 27 class-masks (c_pred 1..9,
intersection via w = 11*yp + yt == 12c, c_true 1..9) are built on the DVE
as plain bf16 is_equal passes (4x mode), and reduced by the TensorEngine:
a ones[128,1] stationary matmul column-sums each mask slab into a
per-class PSUM slice, PSUM-accumulating across slabs and chunks. PSUM
slices live at partitions {0,32,64,96} (tile_position col-groups) x 7
bank slots. A final DVE reduce drains PSUM to a [128, 8] tile, DMA'd out;
the host does the tiny final dice reduction.

accum_out (TensorScalarPtrReduce) is deliberately NOT used: measured on
HW it is ~10x slower than a plain tensor_scalar pass.
"""

import numpy as np

NUM_CLASSES = 10
N_CORES = 8
SHAPE = (4, 1, 160, 160, 160)
N_SAMPLES = 4
V_TOTAL = 4 * 160 * 160 * 160          # 16_384_000
V_CORE = V_TOTAL // N_CORES            # 2_048_000
P = 128
F = V_CORE // P                        # 16000
NCHUNK = 2
FC = F // NCHUNK                       # 8000
MM_N = 500                             # matmul slab width (psum slot pitch 512)
CHUNK_PLAN = [(0, 2000), (2000, 6000), (8000, 8000)]  # (offset, size) in F cols
N_CLS = 27
N_FOLD = 9                             # masks pre-folded on DVE before the PE

ACT_CLS = {19, 20, 22, 23, 26}

_CACHE = {}


def _build_bass(repeat=1, variant="full"):
    import concourse.bacc as bacc
    import concourse.mybir as mybir
    import concourse.tile as tile

    nc = bacc.Bacc(None, target_bir_lowering=False)
    yp_d = nc.dram_tensor("yp", [P, F], mybir.dt.int32, kind="ExternalInput")
    yt_d = nc.dram_tensor("yt", [P, F], mybir.dt.int32, kind="ExternalInput")
    out_d = nc.dram_tensor("out", [P, 8], mybir.dt.float32, kind="ExternalOutput")

    eq = mybir.AluOpType.is_equal
    bf16 = mybir.dt.bfloat16
    f32 = mybir.dt.float32
    n_slabs = FC // MM_N

    with tile.TileContext(nc) as tc:
        with (
            tc.tile_pool(name="io", bufs=2) as io_pool,
            tc.tile_pool(name="work", bufs=2) as work_pool,
            tc.tile_pool(name="mask", bufs=3) as mask_pool,
            tc.tile_pool(name="fold", bufs=2) as fold_pool,
            tc.tile_pool(name="act", bufs=2) as act_pool,
            tc.tile_pool(name="act1", bufs=1) as act1_pool,
            tc.tile_pool(name="acc", bufs=1) as acc_pool,
            tc.tile_pool(name="psum", bufs=1, space="PSUM") as psum_pool,
        ):
            ones = acc_pool.tile([P, 1], bf16)
            nc.gpsimd.memset(ones[:], 1.0)
            sqb = acc_pool.tile([P, N_CLS], f32)
            for i2 in range(N_CLS):
                if i2 in ACT_CLS:
                    cv = (i2 - 18 + 1) if i2 >= 18 else 0
                    nc.gpsimd.memset(sqb[:, i2:i2 + 1], -float(cv))
            acc = acc_pool.tile([P, 8], f32)
            nc.gpsimd.memset(acc[:], 0.0)
            # one psum tile spanning 7 banks; class i uses
            # [32*(i%4) : 32*(i%4)+1, 512*(i//4) : 512*(i//4)+MM_N]
            psum = psum_pool.tile([P, 7 * 512], f32)

            chunk_plan = CHUNK_PLAN
            n_chunks = len(chunk_plan)
            for k0 in range(n_chunks * repeat):
                k = k0 % n_chunks
                off, FCk = chunk_plan[k]
                ypc = io_pool.tile([P, FC], bf16, tag="ypc")
                ytc = io_pool.tile([P, FC], bf16, tag="ytc")
                ypc = ypc[:, :FCk]
                ytc = ytc[:, :FCk]
                # SWDGE dma casts int32 -> bf16 during the transfer
                nc.gpsimd.dma_start(ypc[:], yp_d[:, off:off + FCk])
                nc.gpsimd.dma_start(ytc[:], yt_d[:, off:off + FCk])

                # w = 11*yp + yt in [0, 109]; intersection_c == count(w == 12c)
                w = work_pool.tile([P, FC], bf16, tag="w")
                w = w[:, :FCk]
                nc.vector.scalar_tensor_tensor(
                    out=w[:], in0=ypc[:], scalar=11.0, in1=ytc[:],
                    op0=mybir.AluOpType.mult, op1=mybir.AluOpType.add,
                )

                jobs = (
                    [(ypc, float(c)) for c in range(1, 10)]
                    + [(w, float(12 * c)) for c in range(1, 10)]
                    + [(ytc, float(c)) for c in range(1, 10)]
                )
                for i, (src, cval) in enumerate(jobs):
                    if variant == "pe_only":
                        if k0 == 0 and i == 0:
                            pe_mask = acc_pool.tile([P, FC], bf16)
                            _CACHE["pe_mask"] = pe_mask
                            nc.vector.tensor_scalar(
                                out=_CACHE["pe_mask"][:], in0=src[:],
                                scalar1=cval, scalar2=0.0,
                                op0=eq, op1=mybir.AluOpType.add,
                            )
                        mask = _CACHE["pe_mask"][:, :FCk]
                    elif i in ACT_CLS and variant == "full":
                        # ScalarE 2-pass mask: relu(1 - (v - c)^2)
                        at = act1_pool.tile([P, FC], bf16, tag="actt")
                        at = at[:, :FCk]
                        nc.scalar.activation(
                            out=at[:], in_=src[:],
                            func=mybir.ActivationFunctionType.Square,
                            bias=sqb[:, i:i + 1],
                        )
                        mask = act_pool.tile([P, FC], bf16, tag="actm")
                        mask = mask[:, :FCk]
                        nc.scalar.activation(
                            out=mask[:], in_=at[:],
                            func=mybir.ActivationFunctionType.Relu,
                            bias=1.0, scale=-1.0,
                        )
                    else:
                        mask = mask_pool.tile([P, FC], bf16, tag="mask")
                        mask = mask[:, :FCk]
                        nc.vector.tensor_scalar(
                            out=mask[:], in0=src[:], scalar1=cval, scalar2=0.0,
                            op0=eq, op1=mybir.AluOpType.add,
                        )
                    if variant == "dve_only":
                        if k0 == NCHUNK * repeat - 1 and i == N_CLS - 1:
                            nc.tensor.matmul(
                                psum[0:1, 0:MM_N], ones[:], mask[:, 0:MM_N],
                                start=True, stop=True, tile_position=(0, 0),
                            )
                        continue
                    if (i % 4 == 1 or i in (3, 7)) and i not in ACT_CLS:
                        # DVE folds mask halves ({0,1,2} values) to halve
                        # the PE stream for this class
                        fm = fold_pool.tile([P, FC // 2], bf16, tag="fmask")
                        fm = fm[:, :FCk // 2]
                        nc.vector.tensor_tensor(
                            out=fm[:], in0=mask[:, :FCk // 2],
                            in1=mask[:, FCk // 2:], op=mybir.AluOpType.add,
                        )
                        feed, fw = fm, FCk // 2
                    else:
                        feed, fw = mask, FCk
                    grp, slot = i % 4, i // 4
                    prow = 32 * grp
                    for s in range(fw // MM_N):
                        nc.tensor.matmul(
                            psum[prow:prow + 1,
                                 512 * slot:512 * slot + MM_N],
                            ones[:],
                            feed[:, s * MM_N:(s + 1) * MM_N],
                            start=(k == 0 and s == 0),
                            stop=(k == n_chunks - 1 and s == fw // MM_N - 1),
                            tile_position=(0, prow),
                        )

            # drain PSUM: per partition-group, reduce [1, 7, MM_N] -> [1, 7]
            for grp in range(4):
                prow = 32 * grp
                view = psum[prow:prow + 1, :].rearrange(
                    "p (b n) -> p b n", n=512
                )[:, :, 0:MM_N]
                nc.vector.tensor_reduce(
                    out=acc[prow:prow + 1, 0:7], in_=view,
                    axis=mybir.AxisListType.X, op=mybir.AluOpType.add,
                )
            nc.sync.dma_start(out_d[:], acc[:])
    nc.finalize()
    return nc


def _get_built():
    if "nc" not in _CACHE:
        _CACHE["nc"] = _build_bass()
    return _CACHE["nc"]


def _decode_counts(raw):
    """raw: [P, 8] per-core output -> (cp[9], ct[9], inter[9])."""
    vals = np.zeros(N_CLS, np.float64)
    for i in range(N_CLS):
        vals[i] = raw[32 * (i % 4), i // 4]
    return vals[0:9], vals[18:27], vals[9:18]


def _host_finish(per_core_raw):
    cp = np.zeros((N_SAMPLES, 9), np.float64)
    ct = np.zeros((N_SAMPLES, 9), np.float64)
    it = np.zeros((N_SAMPLES, 9), np.float64)
    cores_per_sample = N_CORES // N_SAMPLES
    for core, raw in enumerate(per_core_raw):
        s = core // cores_per_sample
        a, b, c = _decode_counts(raw)
        cp[s] += a
        ct[s] += b
        it[s] += c
    denom = cp + ct
    nonzero = denom > 0
    denom_safe = np.where(nonzero, denom, 1.0)
    dice_terms = np.where(nonzero, 2.0 * it / denom_safe, 0.0)
    weight = ct / ct.sum(-1, keepdims=True) / N_SAMPLES
    loss = 1.0 - np.sum(np.where(nonzero, weight, 0.0) * dice_terms)
    return np.array(loss, dtype=np.float32)


def _make_in_maps(y_pred, y_true):
    yp = np.ascontiguousarray(np.asarray(y_pred).reshape(-1)).astype(
        np.int32, copy=False
    )
    yt = np.ascontiguousarray(np.asarray(y_true).reshape(-1)).astype(
        np.int32, copy=False
    )
    in_maps = []
    for core in range(N_CORES):
        sl = slice(core * V_CORE, (core + 1) * V_CORE)
        in_maps.append({
            "yp": yp[sl].reshape(P, F),
            "yt": yt[sl].reshape(P, F),
        })
    return in_maps


def _run(in_maps, **kw):
    from concourse.bass_utils import run_bass_kernel_spmd

    nc = _get_built()
    res = run_bass_kernel_spmd(nc, in_maps, core_ids=list(range(N_CORES)), **kw)
    per_core = [r["out"] for r in res.results]
    return per_core, res


def kernel(y_pred, y_true):
    per_core, _ = _run(_make_in_maps(y_pred, y_true))
    return _host_finish(per_core)


if __name__ == "__main__":
    rng = np.random.default_rng(0)
    a = rng.integers(0, 10, SHAPE, dtype=np.int32)
    b = rng.integers(0, 10, SHAPE, dtype=np.int32)
    print(kernel(a, b))



# revision 2
# speedup vs baseline: 1.1538x; 1.1538x over previous
"""Trainium2 Bass kernel v3 for DiceFromLabelsLoss (histogram binning).

Per core (1/8 of the flattened voxels, [128, 16000] after reshape):
27 "streams" are column-summed by the TensorEngine into PSUM:
  - 9 intersection masks: is_equal(w, 12c), w = 11*yp + yt  (DVE, 4x)
  - marginal class equations for yp/yt split between DVE is_equal
    singletons and ScalarE Sign cumulative passes (1 pass each)
  - 2 raw-data streams (sum of labels) -- free equations
Class 9 and class 0 of each marginal histogram are recovered on the host
from sum(v) and N. Each stream's matmul uses a one-hot [128,7] station at
col-group g = stream%4, row j = stream//4, so its total lands on PSUM
partition 32g+j; matmuls round-robin the 4 col-groups (3.4x concurrent).
All input DMAs (SWDGE int32->bf16 cast) are issued upfront with a ramped
chunk plan so compute starts ~10us in; constants arrive via one HWDGE DMA.
Drain: 4 tiny tensor_reduces -> [128,1] f32 out; host solves the linear
systems and computes the dice loss in f64.
"""

import numpy as np

NUM_CLASSES = 10
N_CORES = 8
SHAPE = (4, 1, 160, 160, 160)
N_SAMPLES = 4
V_TOTAL = 4 * 160 * 160 * 160          # 16_384_000
V_CORE = V_TOTAL // N_CORES            # 2_048_000
P = 128
F = V_CORE // P                        # 16000
MM_N = 500                             # matmul slab width
CHUNKS = [4000, 4000, 4000, 4000]
assert sum(CHUNKS) == F

# Marginal equation split (per tensor): classes with DVE is_equal
# singletons, classes covered by ScalarE Sign cumulative anchors.
DVE_P = [1, 2, 3, 4, 5]
SC_P = [6, 7, 8]
DVE_T = [1, 2, 3, 4, 5]
SC_T = [6, 7, 8]

# Stream table: (kind, param). Order = PE consumption order; slot g=i%4,
# j=i//4. Early streams depend on fewer inputs (yp-only first).
STREAMS = (
    [("rawp", 0)]
    + [("eqp", c) for c in DVE_P]
    + [("sgp", c) for c in SC_P]
    + [("rawt", 0)]
    + [("eqt", c) for c in DVE_T]
    + [("sgt", c) for c in SC_T]
    + [("eqw", c) for c in range(1, 10)]
)
assert len(STREAMS) == 27

_CACHE = {}


def _const_arrays():
    """Host-precomputed constants: stations [128, 49] bf16 (7 one-hot
    [128,7] stations), biases [128, 10] f32 (Sign anchors -(c-0.5))."""
    import ml_dtypes

    st = np.zeros((P, 7, 7), np.float32)
    for j in range(7):
        st[:, j, j] = 1.0
    st = st.reshape(P, 49).astype(ml_dtypes.bfloat16)
    bias = np.zeros((P, 10), np.float32)
    for c in range(1, 10):
        bias[:, c] = -(float(c) - 0.5)
    return st, bias


def _build_bass(repeat=1, gpsimd_w=False, dma_in_loop=False):
    import concourse.bacc as bacc
    import concourse.mybir as mybir
    import concourse.tile as tile

    nc = bacc.Bacc(None, target_bir_lowering=False)
    yp_d = nc.dram_tensor("yp", [P, F], mybir.dt.int32, kind="ExternalInput")
    yt_d = nc.dram_tensor("yt", [P, F], mybir.dt.int32, kind="ExternalInput")
    st_d = nc.dram_tensor("cst_st", [P, 49], mybir.dt.bfloat16,
                          kind="ExternalInput")
    bias_d = nc.dram_tensor("cst_bias", [P, 10], mybir.dt.float32,
                            kind="ExternalInput")
    out_d = nc.dram_tensor("out", [P, 1], mybir.dt.float32,
                           kind="ExternalOutput")

    eq = mybir.AluOpType.is_equal
    add = mybir.AluOpType.add
    mult = mybir.AluOpType.mult
    bf16 = mybir.dt.bfloat16
    f32 = mybir.dt.float32
    Sign = mybir.ActivationFunctionType.Sign

    n_chunks = len(CHUNKS)
    offs = np.cumsum([0] + CHUNKS).tolist()

    with tile.TileContext(nc) as tc:
        with (
            tc.tile_pool(name="const", bufs=1) as const_pool,
            tc.tile_pool(name="io", bufs=1) as io_pool,
            tc.tile_pool(name="wt", bufs=4) as w_pool,
            tc.tile_pool(name="mask", bufs=6) as mask_pool,
            tc.tile_pool(name="smask", bufs=4) as smask_pool,
            tc.tile_pool(name="psum", bufs=1, space="PSUM") as psum_pool,
        ):
            # ---- all input DMAs upfront (SWDGE queue saturated from t=0)
            io_tiles = []
            for k in range(n_chunks):
                fc = CHUNKS[k]
                ypc = io_pool.tile([P, fc], bf16, tag=f"ypc{k}")
                ytc = io_pool.tile([P, fc], bf16, tag=f"ytc{k}")
                if not dma_in_loop:
                    nc.gpsimd.dma_start(ypc[:], yp_d[:, offs[k]:offs[k] + fc])
                    nc.gpsimd.dma_start(ytc[:], yt_d[:, offs[k]:offs[k] + fc])
                io_tiles.append((ypc, ytc))
            # constants via HWDGE (no Q7 involvement)
            st_t = const_pool.tile([P, 49], bf16, tag="st")
            bias = const_pool.tile([P, 10], f32, tag="bias")
            nc.sync.dma_start(st_t[:], st_d[:])
            nc.sync.dma_start(bias[:], bias_d[:])
            stations = [st_t[:, 7 * j:7 * j + 7] for j in range(7)]
            acc = const_pool.tile([P, 1], f32, tag="acc")
            nc.gpsimd.memset(acc[:], 0.0)
            psum = psum_pool.tile([P, 512], f32)

            group_started = [False] * 4
            mm_of_group = [0, 0, 0, 0]
            n_group_mms = [0, 0, 0, 0]
            for i in range(27):
                n_group_mms[i % 4] += (F // MM_N) * repeat

            def emit_mms(blocks, fc):
                for s in range(fc // MM_N):
                    for (i, feed) in blocks:
                        g, j = i % 4, i // 4
                        prow = 32 * g
                        start = not group_started[g]
                        group_started[g] = True
                        mm_of_group[g] += 1
                        stop = mm_of_group[g] == n_group_mms[g]
                        nc.tensor.matmul(
                            psum[prow:prow + 7, 0:MM_N],
                            stations[j],
                            feed[:, s * MM_N:(s + 1) * MM_N],
                            start=start, stop=stop,
                            tile_position=(0, prow),
                        )

            for k0 in range(n_chunks * repeat):
                k = k0 % n_chunks
                fc = CHUNKS[k]
                ypc, ytc = io_tiles[k]
                if dma_in_loop:
                    nc.gpsimd.dma_start(ypc[:], yp_d[:, offs[k]:offs[k] + fc])
                    nc.gpsimd.dma_start(ytc[:], yt_d[:, offs[k]:offs[k] + fc])
                blocks = []

                def flush():
                    if blocks:
                        emit_mms(list(blocks), fc)
                        blocks.clear()

                for i, (kind, c) in enumerate(STREAMS):
                    if kind == "rawp":
                        blocks.append((i, ypc))
                    elif kind == "rawt":
                        blocks.append((i, ytc))
                    elif kind == "eqp" or kind == "eqt":
                        src = ypc if kind == "eqp" else ytc
                        m = mask_pool.tile([P, fc], bf16, tag="m")
                        nc.vector.tensor_scalar(
                            out=m[:], in0=src[:], scalar1=float(c),
                            scalar2=0.0, op0=eq, op1=add,
                        )
                        blocks.append((i, m))
                    elif kind == "sgp" or kind == "sgt":
                        src = ypc if kind == "sgp" else ytc
                        m = smask_pool.tile([P, fc], bf16, tag="sm")
                        nc.scalar.activation(
                            out=m[:], in_=src[:], func=Sign,
                            bias=bias[:, c:c + 1], scale=1.0,
                        )
                        blocks.append((i, m))
                    elif kind == "eqw":
                        if c == 1:
                            w11 = w_pool.tile([P, fc], bf16, tag="w11")
                            nc.vector.tensor_scalar(
                                out=w11[:], in0=ypc[:], scalar1=11.0,
                                scalar2=0.0, op0=mult, op1=add,
                            )
                            w = w_pool.tile([P, fc], bf16, tag="w")
                            eng = nc.gpsimd if gpsimd_w else nc.vector
                            eng.tensor_tensor(
                                out=w[:], in0=w11[:], in1=ytc[:], op=add,
                            )
                        m = mask_pool.tile([P, fc], bf16, tag="m")
                        nc.vector.tensor_scalar(
                            out=m[:], in0=w[:], scalar1=float(12 * c),
                            scalar2=0.0, op0=eq, op1=add,
                        )
                        blocks.append((i, m))
                    if len(blocks) == 4:
                        flush()
                flush()

            # drain: per col-group reduce [7, 500] -> [7, 1]
            for g in range(4):
                prow = 32 * g
                nc.vector.tensor_reduce(
                    out=acc[prow:prow + 7, 0:1],
                    in_=psum[prow:prow + 7, 0:MM_N],
                    axis=mybir.AxisListType.X, op=add,
                )
            nc.sync.dma_start(out_d[:], acc[:])
    nc.finalize()
    return nc


def _get_built():
    if "nc" not in _CACHE:
        _CACHE["nc"] = _build_bass()
    return _CACHE["nc"]


def _make_in_maps(y_pred, y_true):
    yp = np.ascontiguousarray(np.asarray(y_pred).reshape(-1)).astype(
        np.int32, copy=False)
    yt = np.ascontiguousarray(np.asarray(y_true).reshape(-1)).astype(
        np.int32, copy=False)
    st, bias = _const_arrays()
    in_maps = []
    for core in range(N_CORES):
        sl = slice(core * V_CORE, (core + 1) * V_CORE)
        in_maps.append({
            "yp": yp[sl].reshape(P, F),
            "yt": yt[sl].reshape(P, F),
            "cst_st": st,
            "cst_bias": bias,
        })
    return in_maps


def _decode_streams(raw):
    vals = {}
    for i in range(27):
        g, j = i % 4, i // 4
        vals[i] = float(raw[32 * g + j, 0])
    return vals


def _solve_marginal(n_total, singles, cums, raw_sum):
    rows, rhs = [], []
    r = np.ones(10)
    rows.append(r); rhs.append(n_total)
    r = np.arange(10, dtype=np.float64)
    rows.append(r); rhs.append(raw_sum)
    for c, v in singles.items():
        r = np.zeros(10); r[c] = 1.0
        rows.append(r); rhs.append(v)
    for c, v in cums.items():
        r = np.zeros(10); r[c:] = 1.0
        rows.append(r); rhs.append(v)
    A = np.stack(rows); b = np.array(rhs, dtype=np.float64)
    h, *_ = np.linalg.lstsq(A, b, rcond=None)
    return h


def _host_finish(per_core_raw):
    n_per_sample = 2 * V_CORE
    cores_per_sample = N_CORES // N_SAMPLES
    hp = np.zeros((N_SAMPLES, 10))
    ht = np.zeros((N_SAMPLES, 10))
    it = np.zeros((N_SAMPLES, 10))
    for s in range(N_SAMPLES):
        vals = None
        for core in range(s * cores_per_sample, (s + 1) * cores_per_sample):
            v = _decode_streams(per_core_raw[core])
            vals = v if vals is None else {i: vals[i] + v[i] for i in v}
        sing_p, cum_p, sing_t, cum_t = {}, {}, {}, {}
        raw_p = raw_t = 0.0
        for i, (kind, c) in enumerate(STREAMS):
            x = vals[i]
            if kind == "rawp":
                raw_p = x
            elif kind == "rawt":
                raw_t = x
            elif kind == "eqp":
                sing_p[c] = x
            elif kind == "eqt":
                sing_t[c] = x
            elif kind == "sgp":
                cum_p[c] = (x + n_per_sample) / 2.0
            elif kind == "sgt":
                cum_t[c] = (x + n_per_sample) / 2.0
            elif kind == "eqw":
                it[s, c] = x
        hp[s] = _solve_marginal(n_per_sample, sing_p, cum_p, raw_p)
        ht[s] = _solve_marginal(n_per_sample, sing_t, cum_t, raw_t)

    cp = hp[:, 1:]
    ct = ht[:, 1:]
    inter = it[:, 1:]
    denom = cp + ct
    nonzero = denom > 0
    denom_safe = np.where(nonzero, denom, 1.0)
    dice_terms = np.where(nonzero, 2.0 * inter / denom_safe, 0.0)
    weight = ct / ct.sum(-1, keepdims=True) / N_SAMPLES
    loss = 1.0 - np.sum(np.where(nonzero, weight, 0.0) * dice_terms)
    return np.array(loss, dtype=np.float32)


def _run(in_maps, **kw):
    from concourse.bass_utils import run_bass_kernel_spmd

    nc = _get_built()
    res = run_bass_kernel_spmd(nc, in_maps, core_ids=list(range(N_CORES)),
                               **kw)
    per_core = [r["out"] for r in res.results]
    return per_core, res


def kernel(y_pred, y_true):
    per_core, _ = _run(_make_in_maps(y_pred, y_true))
    return _host_finish(per_core)


if __name__ == "__main__":
    rng = np.random.default_rng(0)
    a = rng.integers(0, 10, SHAPE, dtype=np.int32)
    b = rng.integers(0, 10, SHAPE, dtype=np.int32)
    print(kernel(a, b))
